# revision 32
# baseline (speedup 1.0000x reference)
"""Trainium2 Bass kernel for DNN-IVA (15-iteration ISS + per-frame MLP mask net).

Sharding: data-parallel over B (4 ways) x T (2 ways) = 8 cores.
Each core handles one batch element's half of the time frames.  The only
cross-core coupling is the per-iteration reduction over T (the ISS statistics),
reformulated so each iteration needs exactly ONE tiny pair-AllReduce (20 KB).

Math reformulation (validated vs reference): per iteration, both ISS source
steps depend on the big (C,F,T) tensors only through 8 per-(f) reductions
  q0..q3 = sum_t w_c * |Y_i|^2,   q4..q7 = sum_t w_c * Re/Im(Y1 conj(Y0))
after which the source-step updates collapse to a per-frequency 2x2 complex
matrix A applied to the two channel rows:  Y'' = A Y.

On-chip layout: f on partitions (5 chunks of 128; chunk 4 has 1 valid lane),
t on the free dimension.  Products+reductions fused via tensor_tensor_reduce;
the 2x2 apply uses scalar_tensor_tensor with per-partition coefficient APs.

Host I/O path: all per-core inputs are packed into ONE fp16 buffer (one
device_put over the axon tunnel), the output is ONE fp16 buffer per core
(one fetch).  The jitted shard_map executable is cached across calls, and
the previous call's (fully-overwritten) output buffer is donated back as
the next call's output allocation so no zero-buffer is ever transferred.
"""

import os
import sys

import numpy as np

import concourse.bass as bass
import concourse.tile as tile
from concourse import bacc, mybir, masks

B, T, C, F, U = 4, 1000, 2, 513, 256
N_ITER = 15
EPS = 1e-6
N_CORES = 8
TSPLIT = 2
TL = T // TSPLIT          # 500 local frames per core
NJ = 5                    # f chunks of 128 (last has 1 valid row)
FSZ = [128, 128, 128, 128, 1]
TT_SIZES = [128, 128, 128, 116]   # t tiles covering TL=500 for load/store
FP = mybir.dt.float32
F16 = mybir.dt.float16
BF = mybir.dt.bfloat16
AL = mybir.AluOpType
AF = mybir.ActivationFunctionType

# packed fp16 input layout (per core).  The mask-net weights are sharded
# 8 ways across cores and AllGather'ed on device (cheap NeuronLink hop)
# instead of being broadcast over the slow host->device tunnel.
NXV = TL * C * F                  # 513000 elems per plane
NW = F * U                        # 131328
WTOT = 2 * NW + U + F             # 263425 packed weight elems


def _wsh_for(g):
    """Per-core weight-shard elems for a g-core group (mult of 4)."""
    return ((WTOT + g - 1) // g + 3) // 4 * 4


def _per_for(g):
    return 2 * NXV + _wsh_for(g)


WSH = _wsh_for(8)                 # 32932
OFF_XR = 0
OFF_XI = OFF_XR + NXV
OFF_WS = OFF_XI + NXV
PER = OFF_WS + WSH                # fp16 elems per core (8-core layout)
# offsets within the gathered 8*WSH weight buffer
GW1 = 0
GW2 = GW1 + NW
GB1 = GW2 + NW
GB2 = GB1 + U
OLEN = 2 * C * TL * F             # packed output: (p, c, t, f)
# int8 output mode: quantized (p,c,t,f) int8 + per-(f,j,c) fp16 scales tail
OUT8 = os.environ.get("KOUT8", "1") == "1"
NSC = 128 * NJ * C                # shipped scale slots (fp16)
SCB = 2 * NSC                     # scale tail bytes
OLEN8 = OLEN + SCB                # int8 elems per core
MAGIC = 12582912.0                # 1.5*2^23: fp32 round-to-nearest trick

_CACHED = {}


def _fslice(tile_ap, j, cols):
    """AP for f-chunk j of a [128, NJ*TL]-shaped plane (cols=TL), valid lanes only."""
    return tile_ap[0 : FSZ[j], j * cols : (j + 1) * cols]


def _build(g=N_CORES, pairs=None, wg=None):
    """pairs: explicit 2-core collective groups (default: all g//2 pairs).
    wg: weight-shard count (default g; with explicit pairs, 2)."""
    nc = bacc.Bacc("TRN2", target_bir_lowering=False, debug=False,
                   num_devices=g)
    if wg is None:
        wg = g if pairs is None else 2
    pk_d = nc.dram_tensor("pk", [_per_for(wg)], F16, kind="ExternalInput").ap()
    if OUT8:
        po_d = nc.dram_tensor("po", [OLEN8], mybir.dt.int8,
                              kind="ExternalOutput").ap()
    else:
        po_d = nc.dram_tensor("po", [OLEN], F16, kind="ExternalOutput").ap()
    with tile.TileContext(nc) as tc:
        _body(nc, tc, pk_d, po_d, g, pairs, wg)
    nc.compile()
    return nc


def _body(nc, tc, pk_d, po_d, g=N_CORES, cc_pairs=None, wg=None):
    if wg is None:
        wg = g if cc_pairs is None else 2
    if cc_pairs is None:
        cc_pairs = [[2 * i, 2 * i + 1] for i in range(g // 2)]
    PLANE = NJ * TL
    xr_d = pk_d[OFF_XR : OFF_XR + NXV].rearrange("(t c f) -> t c f", c=C, f=F)
    xi_d = pk_d[OFF_XI : OFF_XI + NXV].rearrange("(t c f) -> t c f", c=C, f=F)
    yo_d = po_d[0:OLEN].rearrange("(p c t f) -> p c t f", p=2, c=C, t=TL, f=F)
    with (
        tc.tile_pool(name="state", bufs=1) as st,
        tc.tile_pool(name="scr", bufs=3) as scr,
        tc.tile_pool(name="feat", bufs=3) as featp,
        tc.tile_pool(name="hpool", bufs=2) as hp,
        tc.tile_pool(name="small", bufs=12) as sm,
        tc.tile_pool(name="coef", bufs=2) as cf,
        tc.tile_pool(name="psA", bufs=2, space="PSUM") as psA,
        tc.tile_pool(name="psB", bufs=2, space="PSUM") as psB,
        tc.tile_pool(name="dram", bufs=2, space="DRAM") as dram,
        tc.tile_pool(name="outp", bufs=3) as outp,
    ):
        # ---- persistent state -------------------------------------------
        Y = [[st.tile([128, PLANE], FP, tag=f"Y{c}{p}", name=f"Y{c}{p}") for p in range(2)]
             for c in range(C)]                       # [c][0]=re, [1]=im
        X0 = [st.tile([128, PLANE], FP, tag=f"X0{p}", name=f"X0{p}") for p in range(2)]
        A = [st.tile([128, PLANE], BF, tag=f"a{c}", name=f"a{c}") for c in range(C)]
        Wm = [st.tile([128, PLANE], BF, tag=f"w{c}", name=f"w{c}") for c in range(C)]
        W1t = st.tile([128, NJ * U], FP, tag="W1t", name="W1t")
        W2t = st.tile([128, 2 * F], FP, tag="W2t", name="W2t")
        b1t = st.tile([128, 2], FP, tag="b1t", name="b1t")
        b2t = st.tile([128, NJ], FP, tag="b2t", name="b2t")
        ident = st.tile([128, 128], FP, tag="ident", name="ident")
        id16 = st.tile([128, 128], F16, tag="id16", name="id16")
        S = st.tile([128, 8 * NJ], FP, tag="S", name="S")       # quantity-major
        PB = st.tile([128, 12 * NJ], FP, tag="PB", name="PB")    # projection-back stats

        masks.make_identity(nc, ident[:])
        nc.scalar.copy(id16[:], ident[:])

        # ---- gather weight shards on device, then load ------------------
        wsh_g = _wsh_for(wg)
        wgroups = [list(range(g))] if wg == g else cc_pairs
        wg_i = dram.tile([1, wsh_g], F16, tag="wgi", name="wgi")
        wg_o = dram.tile([1, wg * wsh_g], F16, tag="wgo", name="wgo")
        nc.sync.dma_start(wg_i[:], pk_d[OFF_WS : OFF_WS + wsh_g]
                          .rearrange("(o k) -> o k", o=1))
        nc.gpsimd.collective_compute(
            "AllGather", AL.bypass,
            replica_groups=wgroups,
            ins=[wg_i.opt()], outs=[wg_o.opt()])
        wg = wg_o[:].squeeze(0)
        w1_d = wg[GW1 : GW1 + NW].rearrange("(f u) -> f u", u=U)
        w2_d = wg[GW2 : GW2 + NW].rearrange("(u f) -> u f", f=F)
        b1_d = wg[GB1 : GB1 + U]
        b2_d = wg[GB2 : GB2 + F]

        w1s = st.tile([128, NJ * U], F16, tag="w1s", name="w1s")
        w2s = st.tile([128, 2 * F], F16, tag="w2s", name="w2s")
        b1s = st.tile([128, 2], F16, tag="b1s", name="b1s")
        b2s = st.tile([128, NJ], F16, tag="b2s", name="b2s")
        for j in range(NJ):
            nc.sync.dma_start(w1s[0 : FSZ[j], j * U : (j + 1) * U],
                              w1_d[128 * j : 128 * j + FSZ[j], :])
            nc.sync.dma_start(b2s[0 : FSZ[j], j : j + 1],
                              b2_d[128 * j : 128 * j + FSZ[j]].rearrange("(p o) -> p o", o=1))
        for jc in range(2):
            nc.sync.dma_start(w2s[:, jc * F : (jc + 1) * F],
                              w2_d[128 * jc : 128 * (jc + 1), :])
            nc.sync.dma_start(b1s[:, jc : jc + 1],
                              b1_d[128 * jc : 128 * (jc + 1)].rearrange("(p o) -> p o", o=1))
        nc.scalar.copy(W1t[:], w1s[:])
        nc.scalar.copy(W2t[:], w2s[:])
        nc.scalar.copy(b1t[:], b1s[:])
        nc.scalar.copy(b2t[:], b2s[:])

        # ---- load input planes: (t,f) fp16 tiles -> PE transpose -> (f,t)
        for c in range(C):
            for p, src in ((0, xr_d), (1, xi_d)):
                for ti, th in enumerate(TT_SIZES):
                    it_t = scr.tile([128, F], F16, tag="ld", name="ld", bufs=2)
                    nc.sync.dma_start(it_t[0:th, :], src[ti * 128 : ti * 128 + th, c, :])
                    for j in range(NJ):
                        fj = FSZ[j]
                        ps = psB.tile([128, 128], F16, tag="tp16", name="tp16")
                        nc.tensor.transpose(ps[0:fj, 0:th],
                                            it_t[0:th, 128 * j : 128 * j + fj],
                                            id16[0:th, 0:th])
                        nc.scalar.copy(
                            Y[c][p][0:fj, j * TL + ti * 128 : j * TL + ti * 128 + th],
                            ps[0:fj, 0:th])
        for p in range(2):
            nc.vector.tensor_copy(X0[p][:], Y[0][p][:])

        # ---- helper groups ---------------------------------------------
        def qs(q):            # [128, NJ] AP of quantity q in S
            return S[:, q * NJ : (q + 1) * NJ]

        def mask_phase():
            for c in range(C):
                ph = [psA.tile([128, TL], FP, tag="ph", name="ph") for _ in range(2)]
                for j in range(NJ):
                    fj = FSZ[j]
                    s1 = scr.tile([128, TL], FP, tag="sq", name="sq", bufs=4)
                    s2 = scr.tile([128, TL], FP, tag="sq", name="sq", bufs=4)
                    nc.scalar.activation(s1[0:fj, :], _fslice(Y[c][0], j, TL), AF.Square)
                    nc.scalar.activation(s2[0:fj, :], _fslice(Y[c][1], j, TL), AF.Square)
                    nc.gpsimd.tensor_add(_fslice(A[c], j, TL), s1[0:fj, :], s2[0:fj, :])
                    ft = featp.tile([128, TL], FP, tag="ft", name="ft", bufs=4)
                    nc.scalar.activation(ft[0:fj, :], _fslice(A[c], j, TL), AF.Ln,
                                         bias=1.0)
                    for m in range(2):
                        nc.tensor.matmul(
                            ph[m][:, :],
                            W1t[0:fj, j * U + 128 * m : j * U + 128 * (m + 1)],
                            ft[0:fj, :],
                            start=(j == 0), stop=(j == NJ - 1))
                ht = hp.tile([128, 2 * TL], FP, tag="ht", name="ht")
                for m in range(2):
                    nc.scalar.activation(ht[:, m * TL : (m + 1) * TL], ph[m][:, :],
                                         AF.Tanh, bias=b1t[:, m : m + 1])
                for j in range(NJ):
                    fj = FSZ[j]
                    pm = psB.tile([128, TL], FP, tag="pm", name="pm")
                    for jc in range(2):
                        nc.tensor.matmul(
                            pm[0:fj, :],
                            W2t[:, jc * F + 128 * j : jc * F + 128 * j + fj],
                            ht[:, jc * TL : (jc + 1) * TL],
                            start=(jc == 0), stop=(jc == 1))
                    nc.scalar.activation(_fslice(Wm[c], j, TL), pm[0:fj, :],
                                         AF.Sigmoid, bias=b2t[0:fj, j : j + 1])

        def stats_phase():
            for j in range(NJ):
                fj = FSZ[j]
                y0r, y0i = _fslice(Y[0][0], j, TL), _fslice(Y[0][1], j, TL)
                y1r, y1i = _fslice(Y[1][0], j, TL), _fslice(Y[1][1], j, TL)
                m1 = scr.tile([128, TL], BF, tag="pp", name="pp", bufs=4)
                m2 = scr.tile([128, TL], BF, tag="pp", name="pp", bufs=4)
                pr = scr.tile([128, TL], BF, tag="pr", name="pr", bufs=2)
                nc.vector.tensor_mul(m1[0:fj, :], y1r, y0r)
                nc.vector.tensor_mul(m2[0:fj, :], y1i, y0i)
                nc.vector.tensor_add(pr[0:fj, :], m1[0:fj, :], m2[0:fj, :])
                m3 = scr.tile([128, TL], BF, tag="pp", name="pp", bufs=4)
                m4 = scr.tile([128, TL], BF, tag="pp", name="pp", bufs=4)
                pi = scr.tile([128, TL], BF, tag="pi", name="pi", bufs=2)
                nc.gpsimd.tensor_mul(m3[0:fj, :], y1i, y0r)
                nc.gpsimd.tensor_mul(m4[0:fj, :], y1r, y0i)
                nc.gpsimd.tensor_sub(pi[0:fj, :], m3[0:fj, :], m4[0:fj, :])
                srcs = [(Wm[0], _fslice(A[0], j, TL), 0),
                        (Wm[1], _fslice(A[0], j, TL), 1),
                        (Wm[0], _fslice(A[1], j, TL), 2),
                        (Wm[1], _fslice(A[1], j, TL), 3),
                        (Wm[0], pr[0:fj, :], 4), (Wm[0], pi[0:fj, :], 5),
                        (Wm[1], pr[0:fj, :], 6), (Wm[1], pi[0:fj, :], 7)]
                for wt, src_ap, q in srcs:
                    prod = scr.tile([128, TL], BF, tag="pd", name="pd", bufs=6)
                    eng = nc.vector if q % 2 == 0 else nc.gpsimd
                    eng.tensor_mul(prod[0:fj, :], _fslice(wt, j, TL), src_ap)
                    nc.vector.tensor_reduce(
                        S[0:fj, q * NJ + j : q * NJ + j + 1], prod[0:fj, :],
                        axis=mybir.AxisListType.X, op=AL.add)

        def allreduce(tile_t, ncols):
            bi = dram.tile([128, ncols], FP, tag="cin", name="cin")
            bo = dram.tile([128, ncols], FP, tag="cout", name="cout")
            nc.sync.dma_start(bi[:], tile_t[:, 0:ncols])
            nc.gpsimd.collective_compute(
                "AllReduce", AL.add,
                replica_groups=cc_pairs,
                ins=[bi.opt()], outs=[bo.opt()])
            nc.sync.dma_start(tile_t[:, 0:ncols], bo[:])

        def smalls():
            """Per-(f) coefficient algebra on [128, NJ] tiles."""
            def t():
                return sm.tile([128, NJ], FP, tag="smt", name="smt")

            def c(name):
                return cf.tile([128, NJ], FP, tag=name, name=name)
            invT = 1.0 / float(T)
            d0, r0 = t(), t()
            alpha = c("alpha")
            nc.vector.tensor_scalar(d0[:], qs(0), invT, EPS, AL.mult, AL.max)
            nc.vector.reciprocal(r0[:], d0[:])
            nc.scalar.activation(alpha[:], r0[:], AF.Sqrt)
            d1, r1 = t(), t()
            nc.vector.tensor_scalar(d1[:], qs(1), EPS, None, AL.max)
            nc.vector.reciprocal(r1[:], d1[:])
            vr = t()
            vi, nvr, nvi = c("vi"), c("nvr"), c("nvi")
            nc.vector.tensor_mul(vr[:], qs(6), r1[:])
            nc.vector.tensor_mul(vi[:], qs(7), r1[:])
            nc.vector.tensor_scalar_mul(nvr[:], vr[:], -1.0)
            nc.vector.tensor_scalar_mul(nvi[:], vi[:], -1.0)
            m2, u = t(), t()
            nc.vector.tensor_mul(m2[:], vr[:], vr[:])
            nc.vector.scalar_tensor_tensor(u[:], vi[:], 1.0, vi[:], AL.mult, AL.mult)
            nc.vector.tensor_add(m2[:], m2[:], u[:])
            # den0' = q2 - 2(vr q4 + vi q5) + m2 q0 ; den1' likewise with q6,q7,q1,q3
            def denp(qa, qb, qden, qs11):
                x1, x2, e = t(), t(), t()
                nc.vector.tensor_mul(x1[:], vr[:], qa)
                nc.vector.scalar_tensor_tensor(x2[:], vi[:], 1.0, qb, AL.mult, AL.mult)
                nc.vector.tensor_add(x1[:], x1[:], x2[:])
                nc.vector.tensor_mul(e[:], m2[:], qden)
                o = t()
                nc.vector.scalar_tensor_tensor(o[:], x1[:], -2.0, qs11, AL.mult, AL.add)
                nc.vector.tensor_add(o[:], o[:], e[:])
                return o
            den0p = denp(qs(4), qs(5), qs(0), qs(2))
            den1p = denp(qs(6), qs(7), qs(1), qs(3))
            dm, rdm = t(), t()
            nc.vector.tensor_scalar(dm[:], den0p[:], EPS, None, AL.max)
            nc.vector.reciprocal(rdm[:], dm[:])
            # v1 = alpha*((q4,-q5) - conj(v) q0) / den0p
            v1r, tA, tB = t(), t(), t()
            v1i, nv1r, nv1i = c("v1i"), c("nv1r"), c("nv1i")
            nc.vector.tensor_mul(tA[:], vr[:], qs(0))
            nc.vector.tensor_sub(tA[:], qs(4), tA[:])
            nc.vector.tensor_mul(tA[:], tA[:], alpha[:])
            nc.vector.tensor_mul(v1r[:], tA[:], rdm[:])
            nc.vector.tensor_mul(tB[:], vi[:], qs(0))
            nc.vector.tensor_sub(tB[:], tB[:], qs(5))
            nc.vector.tensor_mul(tB[:], tB[:], alpha[:])
            nc.vector.tensor_mul(v1i[:], tB[:], rdm[:])
            nc.vector.tensor_scalar_mul(nv1r[:], v1r[:], -1.0)
            nc.vector.tensor_scalar_mul(nv1i[:], v1i[:], -1.0)
            db, rb = t(), t()
            beta = c("beta")
            nc.vector.tensor_scalar(db[:], den1p[:], invT, EPS, AL.mult, AL.max)
            nc.vector.reciprocal(rb[:], db[:])
            nc.scalar.activation(beta[:], rb[:], AF.Sqrt)
            return alpha, beta, vi, nvr, nvi, v1i, nv1r, nv1i

        def apply_phase(alpha, beta, vi, nvr, nvi, v1i, nv1r, nv1i):
            for j in range(NJ):
                fj = FSZ[j]
                y0r, y0i = _fslice(Y[0][0], j, TL), _fslice(Y[0][1], j, TL)
                y1r, y1i = _fslice(Y[1][0], j, TL), _fslice(Y[1][1], j, TL)
                def c_(ct):
                    return ct[0:fj, j : j + 1]
                t1 = scr.tile([128, TL], FP, tag="ap", name="ap", bufs=4)
                y1pr = scr.tile([128, TL], FP, tag="y1p", name="y1p")
                nc.vector.scalar_tensor_tensor(t1[0:fj, :], y0r, c_(nvr), y1r,
                                               AL.mult, AL.add)
                nc.vector.scalar_tensor_tensor(y1pr[0:fj, :], y0i, c_(vi), t1[0:fj, :],
                                               AL.mult, AL.add)
                t2 = scr.tile([128, TL], FP, tag="ap", name="ap", bufs=4)
                y1pi = scr.tile([128, TL], FP, tag="y1p", name="y1p")
                nc.vector.scalar_tensor_tensor(t2[0:fj, :], y0i, c_(nvr), y1i,
                                               AL.mult, AL.add)
                nc.vector.scalar_tensor_tensor(y1pi[0:fj, :], y0r, c_(nvi), t2[0:fj, :],
                                               AL.mult, AL.add)
                s1 = scr.tile([128, TL], FP, tag="ap", name="ap", bufs=4)
                s2 = scr.tile([128, TL], FP, tag="ap", name="ap", bufs=4)
                nc.scalar.mul(s1[0:fj, :], y0r, c_(alpha))
                nc.scalar.mul(s2[0:fj, :], y0i, c_(alpha))
                t3 = scr.tile([128, TL], FP, tag="ap", name="ap", bufs=4)
                nc.vector.scalar_tensor_tensor(t3[0:fj, :], y1pr[0:fj, :], c_(nv1r),
                                               s1[0:fj, :], AL.mult, AL.add)
                nc.vector.scalar_tensor_tensor(y0r, y1pi[0:fj, :], c_(v1i),
                                               t3[0:fj, :], AL.mult, AL.add)
                t4 = scr.tile([128, TL], FP, tag="ap", name="ap", bufs=4)
                nc.vector.scalar_tensor_tensor(t4[0:fj, :], y1pi[0:fj, :], c_(nv1r),
                                               s2[0:fj, :], AL.mult, AL.add)
                nc.vector.scalar_tensor_tensor(y0i, y1pr[0:fj, :], c_(nv1i),
                                               t4[0:fj, :], AL.mult, AL.add)
                nc.scalar.mul(y1r, y1pr[0:fj, :], c_(beta))
                nc.scalar.mul(y1i, y1pi[0:fj, :], c_(beta))

        # ---- main loop ---------------------------------------------------
        n_it = int(os.environ.get("KITERS", str(N_ITER)))
        do_cc = os.environ.get("KCC", "1") == "1"
        do_pb = os.environ.get("KPB", "1") == "1"
        do_mask = os.environ.get("KMASK", "1") == "1"
        do_stats = os.environ.get("KSTATS", "1") == "1"
        do_apply = os.environ.get("KAPPLY", "1") == "1"
        for _ in range(n_it):
            if do_mask:
                mask_phase()
            if do_stats:
                stats_phase()
            if do_cc:
                allreduce(S, 8 * NJ)
            if do_apply:
                coefs = smalls()
                apply_phase(*coefs)

        # ---- projection back --------------------------------------------
        for j in ([] if not do_pb else range(NJ)):
            fj = FSZ[j]
            for c in range(C):
                pairs = [(Y[c][0], X0[0]), (Y[c][1], X0[1]),
                         (Y[c][0], X0[1]), (Y[c][1], X0[0]),
                         (Y[c][0], Y[c][0]), (Y[c][1], Y[c][1])]
                for qi, (ta, tb) in enumerate(pairs):
                    q = c * 6 + qi
                    prod = scr.tile([128, TL], FP, tag="pd2", name="pd2", bufs=4)
                    if qi >= 4:
                        nc.scalar.activation(prod[0:fj, :], _fslice(ta, j, TL),
                                             AF.Square)
                    else:
                        eng = nc.vector if qi % 2 == 0 else nc.gpsimd
                        eng.tensor_mul(prod[0:fj, :], _fslice(ta, j, TL),
                                       _fslice(tb, j, TL))
                    nc.vector.tensor_reduce(
                        PB[0:fj, q * NJ + j : q * NJ + j + 1], prod[0:fj, :],
                        axis=mybir.AxisListType.X, op=AL.add)
        if do_pb:
            allreduce(PB, 12 * NJ)

        def pbq(q):
            return PB[:, q * NJ : (q + 1) * NJ]

        for c in ([] if not do_pb else range(C)):
            g = [pbq(c * 6 + i) for i in range(6)]
            numr = sm.tile([128, NJ], FP, tag="pbs", name="pbs")
            numi = sm.tile([128, NJ], FP, tag="pbs", name="pbs")
            den = sm.tile([128, NJ], FP, tag="pbs", name="pbs")
            rc = sm.tile([128, NJ], FP, tag="pbs", name="pbs")
            cr = sm.tile([128, NJ], FP, tag=f"cr{c}", name=f"cr{c}")
            ci = sm.tile([128, NJ], FP, tag=f"ci{c}", name=f"ci{c}")
            nci = sm.tile([128, NJ], FP, tag=f"nci{c}", name=f"nci{c}")
            nc.vector.tensor_add(numr[:], g[0], g[1])
            nc.vector.tensor_sub(numi[:], g[2], g[3])
            nc.vector.tensor_add(den[:], g[4], g[5])
            nc.vector.tensor_scalar(den[:], den[:], EPS, None, AL.max)
            nc.vector.reciprocal(rc[:], den[:])
            nc.vector.tensor_mul(cr[:], numr[:], rc[:])
            nc.vector.tensor_mul(ci[:], numi[:], rc[:])
            nc.vector.tensor_scalar_mul(nci[:], ci[:], -1.0)
            for j in range(NJ):
                fj = FSZ[j]
                ycr, yci = _fslice(Y[c][0], j, TL), _fslice(Y[c][1], j, TL)
                s1 = scr.tile([128, TL], FP, tag="ap", name="ap", bufs=4)
                s2 = scr.tile([128, TL], FP, tag="ap", name="ap", bufs=4)
                tr = scr.tile([128, TL], FP, tag="ap", name="ap", bufs=4)
                nc.scalar.mul(s1[0:fj, :], ycr, cr[0:fj, j : j + 1])
                nc.scalar.mul(s2[0:fj, :], yci, cr[0:fj, j : j + 1])
                # new_re = cr*ycr - ci*yci ; new_im = cr*yci + ci*ycr
                nc.vector.scalar_tensor_tensor(tr[0:fj, :], yci, nci[0:fj, j : j + 1],
                                               s1[0:fj, :], AL.mult, AL.add)
                nc.vector.scalar_tensor_tensor(yci, ycr, ci[0:fj, j : j + 1],
                                               s2[0:fj, :], AL.mult, AL.add)
                nc.vector.tensor_copy(ycr, tr[0:fj, :])

        # ---- write output: transpose back to (t,f), DMA out -------------
        if OUT8:
            # per-(c,f) scales: max |.| over local t of re/im, shipped fp16
            mx = st.tile([128, NJ * C], FP, tag="mx", name="mx")
            sinv = st.tile([128, NJ * C], FP, tag="sinv", name="sinv")
            scf = st.tile([128, NJ * C], F16, tag="scf", name="scf")
            for c in range(C):
                for j in range(NJ):
                    fj = FSZ[j]
                    col = j * C + c
                    a1 = scr.tile([128, TL], FP, tag="ab", name="ab", bufs=4)
                    a2 = scr.tile([128, TL], FP, tag="ab", name="ab", bufs=4)
                    nc.scalar.activation(a1[0:fj, :], _fslice(Y[c][0], j, TL), AF.Abs)
                    nc.scalar.activation(a2[0:fj, :], _fslice(Y[c][1], j, TL), AF.Abs)
                    nc.vector.tensor_max(a1[0:fj, :], a1[0:fj, :], a2[0:fj, :])
                    nc.vector.tensor_reduce(mx[0:fj, col : col + 1], a1[0:fj, :],
                                            axis=mybir.AxisListType.X, op=AL.max)
            nc.vector.tensor_scalar(sinv[:], mx[:], 1e-30, None, AL.max)
            nc.vector.reciprocal(sinv[:], sinv[:])
            nc.vector.tensor_scalar_mul(sinv[:], sinv[:], 127.0)
            nc.vector.tensor_scalar_mul(scf[:], mx[:], 1.0 / 127.0)
            sc_d = po_d[OLEN : OLEN + SCB].bitcast(F16).rearrange(
                "(p k) -> p k", k=NJ * C)
            nc.sync.dma_start(sc_d, scf[:])
            for c in range(C):
                for p in range(2):
                    for ti, th in enumerate(TT_SIZES):
                        ot = outp.tile([128, F], mybir.dt.int8, tag="ot8",
                                       name="ot8")
                        for j in range(NJ):
                            fj = FSZ[j]
                            col = j * C + c
                            qt = scr.tile([128, 128], FP, tag="qt", name="qt",
                                          bufs=4)
                            nc.scalar.mul(
                                qt[0:fj, 0:th],
                                Y[c][p][0:fj, j * TL + ti * 128 : j * TL + ti * 128 + th],
                                sinv[0:fj, col : col + 1])
                            nc.vector.tensor_scalar(qt[0:fj, 0:th], qt[0:fj, 0:th],
                                                    MAGIC, -MAGIC, AL.add, AL.add)
                            ps = psB.tile([128, 128], FP, tag="tp", name="tp")
                            nc.tensor.transpose(ps[0:th, 0:fj], qt[0:fj, 0:th],
                                                ident[0:fj, 0:fj])
                            nc.scalar.copy(ot[0:th, 128 * j : 128 * j + fj],
                                           ps[0:th, 0:fj])
                        nc.sync.dma_start(yo_d[p, c, ti * 128 : ti * 128 + th, :],
                                          ot[0:th, :])
        else:
            for c in range(C):
                for p in range(2):
                    for ti, th in enumerate(TT_SIZES):
                        ot = outp.tile([128, F], F16, tag="ot", name="ot")
                        for j in range(NJ):
                            fj = FSZ[j]
                            ps = psB.tile([128, 128], FP, tag="tp", name="tp")
                            nc.tensor.transpose(
                                ps[0:th, 0:fj],
                                Y[c][p][0:fj, j * TL + ti * 128 : j * TL + ti * 128 + th],
                                ident[0:fj, 0:fj])
                            nc.scalar.copy(ot[0:th, 128 * j : 128 * j + fj],
                                           ps[0:th, 0:fj])
                        nc.sync.dma_start(yo_d[p, c, ti * 128 : ti * 128 + th, :],
                                          ot[0:th, :])


class _Exec:
    """Cached jitted shard_map executor with output-buffer donation."""

    def __init__(self, nc, devices=None, g=N_CORES):
        import jax
        from jax.sharding import Mesh, PartitionSpec, NamedSharding
        from jax.experimental.shard_map import shard_map
        from concourse.bass2jax import (
            _bass_exec_p, install_neuronx_cc_hook, partition_id_tensor,
        )
        import jax.numpy as jnp

        self.jax = jax
        self.np = np
        install_neuronx_cc_hook()
        partition_name = (nc.partition_id_tensor.name
                          if nc.partition_id_tensor else None)
        in_names, out_names, out_avals = [], [], []
        in_len = None
        for alloc in nc.m.functions[0].allocations:
            if not isinstance(alloc, mybir.MemoryLocationSet):
                continue
            name = alloc.memorylocations[0].name
            if alloc.kind == "ExternalInput":
                if name != partition_name:
                    in_names.append(name)
                    in_len = int(alloc.tensor_shape[0])
            elif alloc.kind == "ExternalOutput":
                out_names.append(name)
                out_avals.append(jax.core.ShapedArray(
                    tuple(alloc.tensor_shape), mybir.dt.np(alloc.dtype)))
        assert in_names == ["pk"] and out_names == ["po"], (in_names, out_names)
        n_params = len(in_names)
        n_outs = len(out_avals)
        all_in = list(in_names) + list(out_names)
        if partition_name is not None:
            all_in.append(partition_name)

        def _bdy(*args):
            operands = list(args)
            if partition_name is not None:
                operands.append(partition_id_tensor())
            return tuple(_bass_exec_p.bind(
                *operands,
                out_avals=tuple(out_avals),
                in_names=tuple(all_in),
                out_names=tuple(out_names),
                lowering_input_output_aliases=(),
                sim_require_finite=True,
                sim_require_nnan=True,
                nc=nc,
            ))

        if devices is None:
            devices = jax.devices()[:g]
        assert len(devices) == g
        self.devices = devices
        self.g = g
        self.per = in_len
        mesh = Mesh(np.asarray(devices), ("core",))
        self.shard = NamedSharding(mesh, PartitionSpec("core"))
        in_specs = (PartitionSpec("core"),) * (n_params + n_outs)
        out_specs = (PartitionSpec("core"),) * n_outs
        jitfn = jax.jit(
            shard_map(_bdy, mesh=mesh, in_specs=in_specs, out_specs=out_specs,
                      check_rep=False),
            donate_argnums=tuple(range(n_params, n_params + n_outs)),
            keep_unused=True,
        )
        aval = out_avals[0]
        gshape = (g * aval.shape[0],)
        gdtype = aval.dtype
        try:
            from concourse.bass2jax import fast_dispatch_compile
            pk_spec = jax.ShapeDtypeStruct((g * self.per,), np.float16,
                                           sharding=self.shard)
            do_spec = jax.ShapeDtypeStruct(gshape, gdtype, sharding=self.shard)
            self.sharded = fast_dispatch_compile(
                lambda: jitfn.lower(pk_spec, do_spec).compile())
        except Exception:
            self.sharded = jitfn
        self.mkzeros = jax.jit(lambda: jnp.zeros(gshape, gdtype),
                               out_shardings=self.shard)
        self.prev_out = None

    def run_packed(self, bufs):
        """bufs: list of g per-core np fp16 buffers (len self.per).
        Returns np (g*OLEN8,) int8 (or f16 in non-OUT8 builds)."""
        jax = self.jax
        donate = self.prev_out if self.prev_out is not None else self.mkzeros()
        shards = [jax.device_put(bufs[k], self.devices[k])
                  for k in range(self.g)]
        pk_dev = jax.make_array_from_single_device_arrays(
            (self.g * self.per,), self.shard, shards)
        (out,) = self.sharded(pk_dev, donate)
        try:
            out.copy_to_host_async()
        except Exception:
            pass
        res = np.asarray(out)
        self.prev_out = out
        return res

    def run(self, inputs):
        """Pack per-core fp16 buffers, pipelining each device_put with the
        next core's packing.  Returns np (N_CORES*OLEN,) fp16."""
        jax = self.jax
        donate = self.prev_out if self.prev_out is not None else self.mkzeros()
        data_real = np.asarray(inputs["data_real"]).reshape(N_CORES, TL, C, F)
        data_imag = np.asarray(inputs["data_imag"]).reshape(N_CORES, TL, C, F)
        wflat = np.empty(8 * WSH, np.float16)
        wflat[GW1 : GW1 + NW] = np.asarray(inputs["W1"]).reshape(NW)
        wflat[GW2 : GW2 + NW] = np.asarray(inputs["W2"]).reshape(NW)
        wflat[GB1 : GB1 + U] = np.asarray(inputs["b1"])
        wflat[GB2 : GB2 + F] = np.asarray(inputs["b2"])
        wflat[GB2 + F :] = 0
        wsh = wflat.reshape(N_CORES, WSH)
        shards = []
        for k in range(N_CORES):
            buf = np.empty(PER, np.float16)
            buf[OFF_XR : OFF_XR + NXV].reshape(TL, C, F)[...] = data_real[k]
            buf[OFF_XI : OFF_XI + NXV].reshape(TL, C, F)[...] = data_imag[k]
            buf[OFF_WS : OFF_WS + WSH] = wsh[k]
            shards.append(jax.device_put(buf, self.devices[k]))
        pk_dev = jax.make_array_from_single_device_arrays(
            (N_CORES * PER,), self.shard, shards)
        (out,) = self.sharded(pk_dev, donate)
        try:
            out.copy_to_host_async()
        except Exception:
            pass
        # pre-fault the host output array while exec+fetch stream in the
        # background (the async transfer runs on C++ threads regardless)
        outbuf = np.empty((C, B, T, F), dtype=np.complex64)
        outbuf.fill(0)
        res = np.asarray(out)
        self.prev_out = out
        return res, outbuf


def _unpack(res, outbuf=None):
    out = outbuf if outbuf is not None else np.empty((C, B, T, F),
                                                     dtype=np.complex64)
    if OUT8:
        g = res.reshape(N_CORES, OLEN8)
        q = g[:, :OLEN].reshape(B, TSPLIT, 2, C, TL, F)
        sc = np.ascontiguousarray(g[:, OLEN:]).view(np.float16)
        sc = sc.reshape(N_CORES, 128, NJ, C)
        scale = np.empty((N_CORES, C, F), np.float32)
        for j in range(NJ):
            fj = FSZ[j]
            scale[:, :, 128 * j : 128 * j + fj] = \
                sc[:, 0:fj, j, :].transpose(0, 2, 1)
        sv = scale.reshape(B, TSPLIT, C, F)
        for b in range(B):
            for th in range(TSPLIT):
                sl = slice(th * TL, (th + 1) * TL)
                s = sv[b, th][:, None, :]              # (C,1,F)
                np.multiply(q[b, th, 0], s, out=out.real[:, b, sl, :])
                np.multiply(q[b, th, 1], s, out=out.imag[:, b, sl, :])
        return out
    g = res.reshape(B, TSPLIT, 2, C, TL, F)   # b, th, p, c, t, f
    for b in range(B):
        for th in range(TSPLIT):
            sl = slice(th * TL, (th + 1) * TL)
            out.real[:, b, sl, :] = g[b, th, 0]
            out.imag[:, b, sl, :] = g[b, th, 1]
    return out



# ---- two-worker fan-out: each worker owns its own axon connection ------
SZ_D = B * T * C * F * 4            # one f32 data plane in shm
SZ_W1 = F * U * 4
SZ_W2 = U * F * 4
IN_TOTAL = 2 * SZ_D + SZ_W1 + SZ_W2 + U * 4 + F * 4
OUT_OFF = (IN_TOTAL + 63) // 64 * 64
SHM_SZ = OUT_OFF + N_CORES * OLEN8


def _shm_views(buf):
    dr = np.ndarray((B, T, C, F), np.float32, buf, 0)
    di = np.ndarray((B, T, C, F), np.float32, buf, SZ_D)
    o = 2 * SZ_D
    W1v = np.ndarray((F, U), np.float32, buf, o)
    W2v = np.ndarray((U, F), np.float32, buf, o + SZ_W1)
    b1v = np.ndarray((U,), np.float32, buf, o + SZ_W1 + SZ_W2)
    b2v = np.ndarray((F,), np.float32, buf, o + SZ_W1 + SZ_W2 + U * 4)
    outv = np.ndarray((N_CORES * OLEN8,), np.int8, buf, OUT_OFF)
    return dr, di, W1v, W2v, b1v, b2v, outv


def _worker_entry():
    widx = int(os.environ["KW_IDX"])
    shm_name = os.environ["KW_SHM"]
    addr = os.environ["KW_ADDR"]
    key = bytes.fromhex(os.environ["KW_KEY"])
    from multiprocessing import shared_memory, connection
    conn = connection.Client(addr, authkey=key)
    try:
        shm = shared_memory.SharedMemory(name=shm_name)
        import jax
        nc = _build(g=4)
        ex = _Exec(nc, devices=jax.devices()[:4], g=4)
        wsh4 = _wsh_for(4)
        per4 = _per_for(4)
        dr, di, W1v, W2v, b1v, b2v, outv = _shm_views(shm.buf)
        myout = outv[widx * 4 * OLEN8 : (widx + 1) * 4 * OLEN8]

        def one_call():
            wflat = np.zeros(4 * wsh4, np.float16)
            wflat[GW1 : GW1 + NW] = W1v.reshape(NW)
            wflat[GW2 : GW2 + NW] = W2v.reshape(NW)
            wflat[GB1 : GB1 + U] = b1v
            wflat[GB2 : GB2 + F] = b2v
            wshv = wflat.reshape(4, wsh4)
            drw = dr[2 * widx : 2 * widx + 2].reshape(4, TL, C, F)
            diw = di[2 * widx : 2 * widx + 2].reshape(4, TL, C, F)
            bufs = []
            for k in range(4):
                pb = np.empty(per4, np.float16)
                pb[OFF_XR : OFF_XR + NXV].reshape(TL, C, F)[...] = drw[k]
                pb[OFF_XI : OFF_XI + NXV].reshape(TL, C, F)[...] = diw[k]
                pb[OFF_WS : OFF_WS + wsh4] = wshv[k]
                bufs.append(pb)
            myout[:] = ex.run_packed(bufs)

        one_call()                      # warmup: NEFF load on this client
        conn.send(("ready", widx))
        while True:
            msg = conn.recv()
            if msg is None:
                break
            one_call()
            conn.send(msg)
    except Exception as e:  # noqa: BLE001
        try:
            conn.send(("err", widx, repr(e)))
        except Exception:
            pass


def _teardown_workers():
    st = _CACHED.pop("workers", None)
    if not st:
        return
    for c in st["conns"].values():
        try:
            c.send(None)
            c.close()
        except Exception:
            pass
    for p in st["procs"]:
        try:
            p.terminate()
        except Exception:
            pass
    try:
        st["shm"].close()
        st["shm"].unlink()
    except Exception:
        pass


def _init_workers():
    import atexit
    import secrets
    import subprocess
    from multiprocessing import shared_memory, connection
    shm = shared_memory.SharedMemory(create=True, size=SHM_SZ)
    key = secrets.token_bytes(16)
    addr = f"/tmp/kw_{os.getpid()}_{secrets.token_hex(4)}"
    listener = connection.Listener(addr, authkey=key)
    listener._listener._socket.settimeout(480)
    here = os.path.dirname(os.path.abspath(__file__))
    envb = dict(os.environ, KW_SHM=shm.name, KW_ADDR=addr, KW_KEY=key.hex(),
                PYTHONPATH=here + os.pathsep + os.environ.get("PYTHONPATH", ""))
    procs = []
    for w in range(2):
        env = dict(envb, KW_IDX=str(w))
        procs.append(subprocess.Popen(
            [sys.executable, "-c", "import kernel; kernel._worker_entry()"],
            env=env, stdout=subprocess.DEVNULL, stderr=subprocess.DEVNULL))
    st = {"shm": shm, "procs": procs, "conns": {}, "seq": 0}
    st["views"] = _shm_views(shm.buf)
    _CACHED["workers"] = st
    atexit.register(_teardown_workers)
    try:
        pend = []
        for _ in range(2):
            pend.append(listener.accept())
        for c in pend:
            if not c.poll(480):
                raise RuntimeError("worker init timeout")
            msg = c.recv()
            if msg[0] != "ready":
                raise RuntimeError(f"worker failed: {msg}")
            st["conns"][msg[1]] = c
    finally:
        listener.close()
    if len(st["conns"]) != 2:
        raise RuntimeError("workers not ready")


def _kernel_workers(inputs):
    st = _CACHED["workers"]
    dr, di, W1v, W2v, b1v, b2v, outv = st["views"]
    np.copyto(dr, np.asarray(inputs["data_real"], dtype=np.float32))
    np.copyto(di, np.asarray(inputs["data_imag"], dtype=np.float32))
    np.copyto(W1v, np.asarray(inputs["W1"], dtype=np.float32))
    np.copyto(W2v, np.asarray(inputs["W2"], dtype=np.float32))
    np.copyto(b1v, np.asarray(inputs["b1"], dtype=np.float32))
    np.copyto(b2v, np.asarray(inputs["b2"], dtype=np.float32))
    st["seq"] += 1
    seq = st["seq"]
    for c in st["conns"].values():
        c.send(seq)
    outbuf = np.empty((C, B, T, F), dtype=np.complex64)
    outbuf.fill(0)
    for c in st["conns"].values():
        if not c.poll(45):
            raise RuntimeError("worker call timeout")
        r = c.recv()
        if r != seq:
            raise RuntimeError(f"worker error: {r!r}")
    return _unpack(outv, outbuf)


def _kernel_single(inputs):
    if "ex" not in _CACHED:
        _CACHED["nc"] = _build()
        _CACHED["ex"] = _Exec(_CACHED["nc"])
    ex = _CACHED["ex"]
    res, outbuf = ex.run(inputs)
    return _unpack(res, outbuf)


def kernel(**inputs):
    mode = _CACHED.get("mode")
    if mode == "workers":
        try:
            return _kernel_workers(inputs)
        except Exception:
            _teardown_workers()
            _CACHED["mode"] = "single"
            return _kernel_single(inputs)
    if mode == "single":
        return _kernel_single(inputs)
    if os.environ.get("KWORKERS", "1") == "1":
        try:
            _init_workers()
            out = _kernel_workers(inputs)
            _CACHED["mode"] = "workers"
            return out
        except Exception:
            _teardown_workers()
    _CACHED["mode"] = "single"
    return _kernel_single(inputs)


if __name__ == "__main__":
    rng = np.random.default_rng(0)
    ins = {
        "data_real": rng.standard_normal((B, T, C, F), dtype=np.float32),
        "data_imag": rng.standard_normal((B, T, C, F), dtype=np.float32),
        "ilens": np.full((B,), T, dtype=np.int32),
        "W1": rng.standard_normal((F, U), dtype=np.float32) / np.sqrt(F),
        "b1": np.zeros((U,), dtype=np.float32),
        "W2": rng.standard_normal((U, F), dtype=np.float32) / np.sqrt(U),
        "b2": np.zeros((F,), dtype=np.float32),
    }
    out = kernel(**ins)
    print("kernel ran", out.shape, out.dtype, np.abs(out).mean())


# revision 33
# speedup vs baseline: 2.7097x; 2.7097x over previous
"""Trainium2 Bass kernel for DNN-IVA (15-iteration ISS + per-frame MLP mask net).

Sharding: data-parallel over B (4 ways) x T (2 ways) = 8 cores.
Each core handles one batch element's half of the time frames.  The only
cross-core coupling is the per-iteration reduction over T (the ISS statistics),
reformulated so each iteration needs exactly ONE tiny pair-AllReduce (20 KB).

Math reformulation (validated vs reference): per iteration, both ISS source
steps depend on the big (C,F,T) tensors only through 8 per-(f) reductions
  q0..q3 = sum_t w_c * |Y_i|^2,   q4..q7 = sum_t w_c * Re/Im(Y1 conj(Y0))
after which the source-step updates collapse to a per-frequency 2x2 complex
matrix A applied to the two channel rows:  Y'' = A Y.

On-chip layout: f on partitions (5 chunks of 128; chunk 4 has 1 valid lane),
t on the free dimension.  Products+reductions fused via tensor_tensor_reduce;
the 2x2 apply uses scalar_tensor_tensor with per-partition coefficient APs.

Host I/O path: all per-core inputs are packed into ONE fp16 buffer (one
device_put over the axon tunnel), the output is ONE fp16 buffer per core
(one fetch).  The jitted shard_map executable is cached across calls, and
the previous call's (fully-overwritten) output buffer is donated back as
the next call's output allocation so no zero-buffer is ever transferred.
"""

import os

import numpy as np

import concourse.bass as bass
import concourse.tile as tile
from concourse import bacc, mybir, masks

B, T, C, F, U = 4, 1000, 2, 513, 256
N_ITER = 15
EPS = 1e-6
N_CORES = 8
TSPLIT = 2
TL = T // TSPLIT          # 500 local frames per core
NJ = 5                    # f chunks of 128 (last has 1 valid row)
FSZ = [128, 128, 128, 128, 1]
TT_SIZES = [128, 128, 128, 116]   # t tiles covering TL=500 for load/store
FP = mybir.dt.float32
F16 = mybir.dt.float16
BF = mybir.dt.bfloat16
AL = mybir.AluOpType
AF = mybir.ActivationFunctionType

# packed fp16 input layout (per core).  The mask-net weights are sharded
# 8 ways across cores and AllGather'ed on device (cheap NeuronLink hop)
# instead of being broadcast over the slow host->device tunnel.
NXV = TL * C * F                  # 513000 elems per plane
NW = F * U                        # 131328
WTOT = 2 * NW + U + F             # 263425 packed weight elems


def _wsh_for(g):
    """Per-core weight-shard elems for a g-core group (mult of 4)."""
    return ((WTOT + g - 1) // g + 3) // 4 * 4


def _per_for(g):
    return 2 * NXV + _wsh_for(g)


WSH = _wsh_for(8)                 # 32932
OFF_XR = 0
OFF_XI = OFF_XR + NXV
OFF_WS = OFF_XI + NXV
PER = OFF_WS + WSH                # fp16 elems per core (8-core layout)
# offsets within the gathered 8*WSH weight buffer
GW1 = 0
GW2 = GW1 + NW
GB1 = GW2 + NW
GB2 = GB1 + U
OLEN = 2 * C * TL * F             # packed output: (p, c, t, f)
# int8 output mode: quantized (p,c,t,f) int8 + per-(f,j,c) fp16 scales tail
OUT8 = os.environ.get("KOUT8", "1") == "1"
NSC = 128 * NJ * C                # shipped scale slots (fp16)
SCB = 2 * NSC                     # scale tail bytes
OLEN8 = OLEN + SCB                # int8 elems per core
MAGIC = 12582912.0                # 1.5*2^23: fp32 round-to-nearest trick

_CACHED = {}


def _fslice(tile_ap, j, cols):
    """AP for f-chunk j of a [128, NJ*TL]-shaped plane (cols=TL), valid lanes only."""
    return tile_ap[0 : FSZ[j], j * cols : (j + 1) * cols]


def _build(g=N_CORES, pairs=None, wg=None):
    """pairs: explicit 2-core collective groups (default: all g//2 pairs).
    wg: weight-shard count (default g; with explicit pairs, 2)."""
    nc = bacc.Bacc("TRN2", target_bir_lowering=False, debug=False,
                   num_devices=g)
    if wg is None:
        wg = g if pairs is None else 2
    pk_d = nc.dram_tensor("pk", [_per_for(wg)], F16, kind="ExternalInput").ap()
    if OUT8:
        po_d = nc.dram_tensor("po", [OLEN8], mybir.dt.int8,
                              kind="ExternalOutput").ap()
    else:
        po_d = nc.dram_tensor("po", [OLEN], F16, kind="ExternalOutput").ap()
    with tile.TileContext(nc) as tc:
        _body(nc, tc, pk_d, po_d, g, pairs, wg)
    nc.compile()
    return nc


def _body(nc, tc, pk_d, po_d, g=N_CORES, cc_pairs=None, wg=None):
    if wg is None:
        wg = g if cc_pairs is None else 2
    if cc_pairs is None:
        cc_pairs = [[2 * i, 2 * i + 1] for i in range(g // 2)]
    PLANE = NJ * TL
    xr_d = pk_d[OFF_XR : OFF_XR + NXV].rearrange("(t c f) -> t c f", c=C, f=F)
    xi_d = pk_d[OFF_XI : OFF_XI + NXV].rearrange("(t c f) -> t c f", c=C, f=F)
    yo_d = po_d[0:OLEN].rearrange("(p c t f) -> p c t f", p=2, c=C, t=TL, f=F)
    with (
        tc.tile_pool(name="state", bufs=1) as st,
        tc.tile_pool(name="scr", bufs=3) as scr,
        tc.tile_pool(name="feat", bufs=3) as featp,
        tc.tile_pool(name="hpool", bufs=2) as hp,
        tc.tile_pool(name="small", bufs=12) as sm,
        tc.tile_pool(name="coef", bufs=2) as cf,
        tc.tile_pool(name="psA", bufs=2, space="PSUM") as psA,
        tc.tile_pool(name="psB", bufs=2, space="PSUM") as psB,
        tc.tile_pool(name="dram", bufs=2, space="DRAM") as dram,
        tc.tile_pool(name="outp", bufs=3) as outp,
    ):
        # ---- persistent state -------------------------------------------
        Y = [[st.tile([128, PLANE], FP, tag=f"Y{c}{p}", name=f"Y{c}{p}") for p in range(2)]
             for c in range(C)]                       # [c][0]=re, [1]=im
        X0 = [st.tile([128, PLANE], FP, tag=f"X0{p}", name=f"X0{p}") for p in range(2)]
        A = [st.tile([128, PLANE], BF, tag=f"a{c}", name=f"a{c}") for c in range(C)]
        Wm = [st.tile([128, PLANE], BF, tag=f"w{c}", name=f"w{c}") for c in range(C)]
        W1t = st.tile([128, NJ * U], FP, tag="W1t", name="W1t")
        W2t = st.tile([128, 2 * F], FP, tag="W2t", name="W2t")
        b1t = st.tile([128, 2], FP, tag="b1t", name="b1t")
        b2t = st.tile([128, NJ], FP, tag="b2t", name="b2t")
        ident = st.tile([128, 128], FP, tag="ident", name="ident")
        id16 = st.tile([128, 128], F16, tag="id16", name="id16")
        S = st.tile([128, 8 * NJ], FP, tag="S", name="S")       # quantity-major
        PB = st.tile([128, 12 * NJ], FP, tag="PB", name="PB")    # projection-back stats

        masks.make_identity(nc, ident[:])
        nc.scalar.copy(id16[:], ident[:])

        # ---- gather weight shards on device, then load ------------------
        wsh_g = _wsh_for(wg)
        wgroups = [list(range(g))] if wg == g else cc_pairs
        wg_i = dram.tile([1, wsh_g], F16, tag="wgi", name="wgi")
        wg_o = dram.tile([1, wg * wsh_g], F16, tag="wgo", name="wgo")
        nc.sync.dma_start(wg_i[:], pk_d[OFF_WS : OFF_WS + wsh_g]
                          .rearrange("(o k) -> o k", o=1))
        nc.gpsimd.collective_compute(
            "AllGather", AL.bypass,
            replica_groups=wgroups,
            ins=[wg_i.opt()], outs=[wg_o.opt()])
        wg = wg_o[:].squeeze(0)
        w1_d = wg[GW1 : GW1 + NW].rearrange("(f u) -> f u", u=U)
        w2_d = wg[GW2 : GW2 + NW].rearrange("(u f) -> u f", f=F)
        b1_d = wg[GB1 : GB1 + U]
        b2_d = wg[GB2 : GB2 + F]

        w1s = st.tile([128, NJ * U], F16, tag="w1s", name="w1s")
        w2s = st.tile([128, 2 * F], F16, tag="w2s", name="w2s")
        b1s = st.tile([128, 2], F16, tag="b1s", name="b1s")
        b2s = st.tile([128, NJ], F16, tag="b2s", name="b2s")
        for j in range(NJ):
            nc.sync.dma_start(w1s[0 : FSZ[j], j * U : (j + 1) * U],
                              w1_d[128 * j : 128 * j + FSZ[j], :])
            nc.sync.dma_start(b2s[0 : FSZ[j], j : j + 1],
                              b2_d[128 * j : 128 * j + FSZ[j]].rearrange("(p o) -> p o", o=1))
        for jc in range(2):
            nc.sync.dma_start(w2s[:, jc * F : (jc + 1) * F],
                              w2_d[128 * jc : 128 * (jc + 1), :])
            nc.sync.dma_start(b1s[:, jc : jc + 1],
                              b1_d[128 * jc : 128 * (jc + 1)].rearrange("(p o) -> p o", o=1))
        nc.scalar.copy(W1t[:], w1s[:])
        nc.scalar.copy(W2t[:], w2s[:])
        nc.scalar.copy(b1t[:], b1s[:])
        nc.scalar.copy(b2t[:], b2s[:])

        # ---- load input planes: (t,f) fp16 tiles -> PE transpose -> (f,t)
        for c in range(C):
            for p, src in ((0, xr_d), (1, xi_d)):
                for ti, th in enumerate(TT_SIZES):
                    it_t = scr.tile([128, F], F16, tag="ld", name="ld", bufs=2)
                    nc.sync.dma_start(it_t[0:th, :], src[ti * 128 : ti * 128 + th, c, :])
                    for j in range(NJ):
                        fj = FSZ[j]
                        ps = psB.tile([128, 128], F16, tag="tp16", name="tp16")
                        nc.tensor.transpose(ps[0:fj, 0:th],
                                            it_t[0:th, 128 * j : 128 * j + fj],
                                            id16[0:th, 0:th])
                        nc.scalar.copy(
                            Y[c][p][0:fj, j * TL + ti * 128 : j * TL + ti * 128 + th],
                            ps[0:fj, 0:th])
        for p in range(2):
            nc.vector.tensor_copy(X0[p][:], Y[0][p][:])

        # ---- helper groups ---------------------------------------------
        def qs(q):            # [128, NJ] AP of quantity q in S
            return S[:, q * NJ : (q + 1) * NJ]

        def mask_phase():
            for c in range(C):
                ph = [psA.tile([128, TL], FP, tag="ph", name="ph") for _ in range(2)]
                for j in range(NJ):
                    fj = FSZ[j]
                    s1 = scr.tile([128, TL], FP, tag="sq", name="sq", bufs=4)
                    s2 = scr.tile([128, TL], FP, tag="sq", name="sq", bufs=4)
                    nc.scalar.activation(s1[0:fj, :], _fslice(Y[c][0], j, TL), AF.Square)
                    nc.scalar.activation(s2[0:fj, :], _fslice(Y[c][1], j, TL), AF.Square)
                    nc.gpsimd.tensor_add(_fslice(A[c], j, TL), s1[0:fj, :], s2[0:fj, :])
                    ft = featp.tile([128, TL], FP, tag="ft", name="ft", bufs=4)
                    nc.scalar.activation(ft[0:fj, :], _fslice(A[c], j, TL), AF.Ln,
                                         bias=1.0)
                    for m in range(2):
                        nc.tensor.matmul(
                            ph[m][:, :],
                            W1t[0:fj, j * U + 128 * m : j * U + 128 * (m + 1)],
                            ft[0:fj, :],
                            start=(j == 0), stop=(j == NJ - 1))
                ht = hp.tile([128, 2 * TL], FP, tag="ht", name="ht")
                for m in range(2):
                    nc.scalar.activation(ht[:, m * TL : (m + 1) * TL], ph[m][:, :],
                                         AF.Tanh, bias=b1t[:, m : m + 1])
                for j in range(NJ):
                    fj = FSZ[j]
                    pm = psB.tile([128, TL], FP, tag="pm", name="pm")
                    for jc in range(2):
                        nc.tensor.matmul(
                            pm[0:fj, :],
                            W2t[:, jc * F + 128 * j : jc * F + 128 * j + fj],
                            ht[:, jc * TL : (jc + 1) * TL],
                            start=(jc == 0), stop=(jc == 1))
                    nc.scalar.activation(_fslice(Wm[c], j, TL), pm[0:fj, :],
                                         AF.Sigmoid, bias=b2t[0:fj, j : j + 1])

        def stats_phase():
            for j in range(NJ):
                fj = FSZ[j]
                y0r, y0i = _fslice(Y[0][0], j, TL), _fslice(Y[0][1], j, TL)
                y1r, y1i = _fslice(Y[1][0], j, TL), _fslice(Y[1][1], j, TL)
                m1 = scr.tile([128, TL], BF, tag="pp", name="pp", bufs=4)
                m2 = scr.tile([128, TL], BF, tag="pp", name="pp", bufs=4)
                pr = scr.tile([128, TL], BF, tag="pr", name="pr", bufs=2)
                nc.vector.tensor_mul(m1[0:fj, :], y1r, y0r)
                nc.vector.tensor_mul(m2[0:fj, :], y1i, y0i)
                nc.vector.tensor_add(pr[0:fj, :], m1[0:fj, :], m2[0:fj, :])
                m3 = scr.tile([128, TL], BF, tag="pp", name="pp", bufs=4)
                m4 = scr.tile([128, TL], BF, tag="pp", name="pp", bufs=4)
                pi = scr.tile([128, TL], BF, tag="pi", name="pi", bufs=2)
                nc.gpsimd.tensor_mul(m3[0:fj, :], y1i, y0r)
                nc.gpsimd.tensor_mul(m4[0:fj, :], y1r, y0i)
                nc.gpsimd.tensor_sub(pi[0:fj, :], m3[0:fj, :], m4[0:fj, :])
                srcs = [(Wm[0], _fslice(A[0], j, TL), 0),
                        (Wm[1], _fslice(A[0], j, TL), 1),
                        (Wm[0], _fslice(A[1], j, TL), 2),
                        (Wm[1], _fslice(A[1], j, TL), 3),
                        (Wm[0], pr[0:fj, :], 4), (Wm[0], pi[0:fj, :], 5),
                        (Wm[1], pr[0:fj, :], 6), (Wm[1], pi[0:fj, :], 7)]
                for wt, src_ap, q in srcs:
                    prod = scr.tile([128, TL], BF, tag="pd", name="pd", bufs=6)
                    eng = nc.vector if q % 2 == 0 else nc.gpsimd
                    eng.tensor_mul(prod[0:fj, :], _fslice(wt, j, TL), src_ap)
                    nc.vector.tensor_reduce(
                        S[0:fj, q * NJ + j : q * NJ + j + 1], prod[0:fj, :],
                        axis=mybir.AxisListType.X, op=AL.add)

        def allreduce(tile_t, ncols):
            bi = dram.tile([128, ncols], FP, tag="cin", name="cin")
            bo = dram.tile([128, ncols], FP, tag="cout", name="cout")
            nc.sync.dma_start(bi[:], tile_t[:, 0:ncols])
            nc.gpsimd.collective_compute(
                "AllReduce", AL.add,
                replica_groups=cc_pairs,
                ins=[bi.opt()], outs=[bo.opt()])
            nc.sync.dma_start(tile_t[:, 0:ncols], bo[:])

        def smalls():
            """Per-(f) coefficient algebra on [128, NJ] tiles."""
            def t():
                return sm.tile([128, NJ], FP, tag="smt", name="smt")

            def c(name):
                return cf.tile([128, NJ], FP, tag=name, name=name)
            invT = 1.0 / float(T)
            d0, r0 = t(), t()
            alpha = c("alpha")
            nc.vector.tensor_scalar(d0[:], qs(0), invT, EPS, AL.mult, AL.max)
            nc.vector.reciprocal(r0[:], d0[:])
            nc.scalar.activation(alpha[:], r0[:], AF.Sqrt)
            d1, r1 = t(), t()
            nc.vector.tensor_scalar(d1[:], qs(1), EPS, None, AL.max)
            nc.vector.reciprocal(r1[:], d1[:])
            vr = t()
            vi, nvr, nvi = c("vi"), c("nvr"), c("nvi")
            nc.vector.tensor_mul(vr[:], qs(6), r1[:])
            nc.vector.tensor_mul(vi[:], qs(7), r1[:])
            nc.vector.tensor_scalar_mul(nvr[:], vr[:], -1.0)
            nc.vector.tensor_scalar_mul(nvi[:], vi[:], -1.0)
            m2, u = t(), t()
            nc.vector.tensor_mul(m2[:], vr[:], vr[:])
            nc.vector.scalar_tensor_tensor(u[:], vi[:], 1.0, vi[:], AL.mult, AL.mult)
            nc.vector.tensor_add(m2[:], m2[:], u[:])
            # den0' = q2 - 2(vr q4 + vi q5) + m2 q0 ; den1' likewise with q6,q7,q1,q3
            def denp(qa, qb, qden, qs11):
                x1, x2, e = t(), t(), t()
                nc.vector.tensor_mul(x1[:], vr[:], qa)
                nc.vector.scalar_tensor_tensor(x2[:], vi[:], 1.0, qb, AL.mult, AL.mult)
                nc.vector.tensor_add(x1[:], x1[:], x2[:])
                nc.vector.tensor_mul(e[:], m2[:], qden)
                o = t()
                nc.vector.scalar_tensor_tensor(o[:], x1[:], -2.0, qs11, AL.mult, AL.add)
                nc.vector.tensor_add(o[:], o[:], e[:])
                return o
            den0p = denp(qs(4), qs(5), qs(0), qs(2))
            den1p = denp(qs(6), qs(7), qs(1), qs(3))
            dm, rdm = t(), t()
            nc.vector.tensor_scalar(dm[:], den0p[:], EPS, None, AL.max)
            nc.vector.reciprocal(rdm[:], dm[:])
            # v1 = alpha*((q4,-q5) - conj(v) q0) / den0p
            v1r, tA, tB = t(), t(), t()
            v1i, nv1r, nv1i = c("v1i"), c("nv1r"), c("nv1i")
            nc.vector.tensor_mul(tA[:], vr[:], qs(0))
            nc.vector.tensor_sub(tA[:], qs(4), tA[:])
            nc.vector.tensor_mul(tA[:], tA[:], alpha[:])
            nc.vector.tensor_mul(v1r[:], tA[:], rdm[:])
            nc.vector.tensor_mul(tB[:], vi[:], qs(0))
            nc.vector.tensor_sub(tB[:], tB[:], qs(5))
            nc.vector.tensor_mul(tB[:], tB[:], alpha[:])
            nc.vector.tensor_mul(v1i[:], tB[:], rdm[:])
            nc.vector.tensor_scalar_mul(nv1r[:], v1r[:], -1.0)
            nc.vector.tensor_scalar_mul(nv1i[:], v1i[:], -1.0)
            db, rb = t(), t()
            beta = c("beta")
            nc.vector.tensor_scalar(db[:], den1p[:], invT, EPS, AL.mult, AL.max)
            nc.vector.reciprocal(rb[:], db[:])
            nc.scalar.activation(beta[:], rb[:], AF.Sqrt)
            return alpha, beta, vi, nvr, nvi, v1i, nv1r, nv1i

        def apply_phase(alpha, beta, vi, nvr, nvi, v1i, nv1r, nv1i):
            for j in range(NJ):
                fj = FSZ[j]
                y0r, y0i = _fslice(Y[0][0], j, TL), _fslice(Y[0][1], j, TL)
                y1r, y1i = _fslice(Y[1][0], j, TL), _fslice(Y[1][1], j, TL)
                def c_(ct):
                    return ct[0:fj, j : j + 1]
                t1 = scr.tile([128, TL], FP, tag="ap", name="ap", bufs=4)
                y1pr = scr.tile([128, TL], FP, tag="y1p", name="y1p")
                nc.vector.scalar_tensor_tensor(t1[0:fj, :], y0r, c_(nvr), y1r,
                                               AL.mult, AL.add)
                nc.vector.scalar_tensor_tensor(y1pr[0:fj, :], y0i, c_(vi), t1[0:fj, :],
                                               AL.mult, AL.add)
                t2 = scr.tile([128, TL], FP, tag="ap", name="ap", bufs=4)
                y1pi = scr.tile([128, TL], FP, tag="y1p", name="y1p")
                nc.vector.scalar_tensor_tensor(t2[0:fj, :], y0i, c_(nvr), y1i,
                                               AL.mult, AL.add)
                nc.vector.scalar_tensor_tensor(y1pi[0:fj, :], y0r, c_(nvi), t2[0:fj, :],
                                               AL.mult, AL.add)
                s1 = scr.tile([128, TL], FP, tag="ap", name="ap", bufs=4)
                s2 = scr.tile([128, TL], FP, tag="ap", name="ap", bufs=4)
                nc.scalar.mul(s1[0:fj, :], y0r, c_(alpha))
                nc.scalar.mul(s2[0:fj, :], y0i, c_(alpha))
                t3 = scr.tile([128, TL], FP, tag="ap", name="ap", bufs=4)
                nc.vector.scalar_tensor_tensor(t3[0:fj, :], y1pr[0:fj, :], c_(nv1r),
                                               s1[0:fj, :], AL.mult, AL.add)
                nc.vector.scalar_tensor_tensor(y0r, y1pi[0:fj, :], c_(v1i),
                                               t3[0:fj, :], AL.mult, AL.add)
                t4 = scr.tile([128, TL], FP, tag="ap", name="ap", bufs=4)
                nc.vector.scalar_tensor_tensor(t4[0:fj, :], y1pi[0:fj, :], c_(nv1r),
                                               s2[0:fj, :], AL.mult, AL.add)
                nc.vector.scalar_tensor_tensor(y0i, y1pr[0:fj, :], c_(nv1i),
                                               t4[0:fj, :], AL.mult, AL.add)
                nc.scalar.mul(y1r, y1pr[0:fj, :], c_(beta))
                nc.scalar.mul(y1i, y1pi[0:fj, :], c_(beta))

        # ---- main loop ---------------------------------------------------
        n_it = int(os.environ.get("KITERS", str(N_ITER)))
        do_cc = os.environ.get("KCC", "1") == "1"
        do_pb = os.environ.get("KPB", "1") == "1"
        do_mask = os.environ.get("KMASK", "1") == "1"
        do_stats = os.environ.get("KSTATS", "1") == "1"
        do_apply = os.environ.get("KAPPLY", "1") == "1"
        for _ in range(n_it):
            if do_mask:
                mask_phase()
            if do_stats:
                stats_phase()
            if do_cc:
                allreduce(S, 8 * NJ)
            if do_apply:
                coefs = smalls()
                apply_phase(*coefs)

        # ---- projection back --------------------------------------------
        for j in ([] if not do_pb else range(NJ)):
            fj = FSZ[j]
            for c in range(C):
                pairs = [(Y[c][0], X0[0]), (Y[c][1], X0[1]),
                         (Y[c][0], X0[1]), (Y[c][1], X0[0]),
                         (Y[c][0], Y[c][0]), (Y[c][1], Y[c][1])]
                for qi, (ta, tb) in enumerate(pairs):
                    q = c * 6 + qi
                    prod = scr.tile([128, TL], FP, tag="pd2", name="pd2", bufs=4)
                    if qi >= 4:
                        nc.scalar.activation(prod[0:fj, :], _fslice(ta, j, TL),
                                             AF.Square)
                    else:
                        eng = nc.vector if qi % 2 == 0 else nc.gpsimd
                        eng.tensor_mul(prod[0:fj, :], _fslice(ta, j, TL),
                                       _fslice(tb, j, TL))
                    nc.vector.tensor_reduce(
                        PB[0:fj, q * NJ + j : q * NJ + j + 1], prod[0:fj, :],
                        axis=mybir.AxisListType.X, op=AL.add)
        if do_pb:
            allreduce(PB, 12 * NJ)

        def pbq(q):
            return PB[:, q * NJ : (q + 1) * NJ]

        for c in ([] if not do_pb else range(C)):
            g = [pbq(c * 6 + i) for i in range(6)]
            numr = sm.tile([128, NJ], FP, tag="pbs", name="pbs")
            numi = sm.tile([128, NJ], FP, tag="pbs", name="pbs")
            den = sm.tile([128, NJ], FP, tag="pbs", name="pbs")
            rc = sm.tile([128, NJ], FP, tag="pbs", name="pbs")
            cr = sm.tile([128, NJ], FP, tag=f"cr{c}", name=f"cr{c}")
            ci = sm.tile([128, NJ], FP, tag=f"ci{c}", name=f"ci{c}")
            nci = sm.tile([128, NJ], FP, tag=f"nci{c}", name=f"nci{c}")
            nc.vector.tensor_add(numr[:], g[0], g[1])
            nc.vector.tensor_sub(numi[:], g[2], g[3])
            nc.vector.tensor_add(den[:], g[4], g[5])
            nc.vector.tensor_scalar(den[:], den[:], EPS, None, AL.max)
            nc.vector.reciprocal(rc[:], den[:])
            nc.vector.tensor_mul(cr[:], numr[:], rc[:])
            nc.vector.tensor_mul(ci[:], numi[:], rc[:])
            nc.vector.tensor_scalar_mul(nci[:], ci[:], -1.0)
            for j in range(NJ):
                fj = FSZ[j]
                ycr, yci = _fslice(Y[c][0], j, TL), _fslice(Y[c][1], j, TL)
                s1 = scr.tile([128, TL], FP, tag="ap", name="ap", bufs=4)
                s2 = scr.tile([128, TL], FP, tag="ap", name="ap", bufs=4)
                tr = scr.tile([128, TL], FP, tag="ap", name="ap", bufs=4)
                nc.scalar.mul(s1[0:fj, :], ycr, cr[0:fj, j : j + 1])
                nc.scalar.mul(s2[0:fj, :], yci, cr[0:fj, j : j + 1])
                # new_re = cr*ycr - ci*yci ; new_im = cr*yci + ci*ycr
                nc.vector.scalar_tensor_tensor(tr[0:fj, :], yci, nci[0:fj, j : j + 1],
                                               s1[0:fj, :], AL.mult, AL.add)
                nc.vector.scalar_tensor_tensor(yci, ycr, ci[0:fj, j : j + 1],
                                               s2[0:fj, :], AL.mult, AL.add)
                nc.vector.tensor_copy(ycr, tr[0:fj, :])

        # ---- write output: transpose back to (t,f), DMA out -------------
        if OUT8:
            # per-(c,f) scales: max |.| over local t of re/im, shipped fp16
            mx = st.tile([128, NJ * C], FP, tag="mx", name="mx")
            sinv = st.tile([128, NJ * C], FP, tag="sinv", name="sinv")
            scf = st.tile([128, NJ * C], F16, tag="scf", name="scf")
            for c in range(C):
                for j in range(NJ):
                    fj = FSZ[j]
                    col = j * C + c
                    a1 = scr.tile([128, TL], FP, tag="ab", name="ab", bufs=4)
                    a2 = scr.tile([128, TL], FP, tag="ab", name="ab", bufs=4)
                    nc.scalar.activation(a1[0:fj, :], _fslice(Y[c][0], j, TL), AF.Abs)
                    nc.scalar.activation(a2[0:fj, :], _fslice(Y[c][1], j, TL), AF.Abs)
                    nc.vector.tensor_max(a1[0:fj, :], a1[0:fj, :], a2[0:fj, :])
                    nc.vector.tensor_reduce(mx[0:fj, col : col + 1], a1[0:fj, :],
                                            axis=mybir.AxisListType.X, op=AL.max)
            nc.vector.tensor_scalar(sinv[:], mx[:], 1e-30, None, AL.max)
            nc.vector.reciprocal(sinv[:], sinv[:])
            nc.vector.tensor_scalar_mul(sinv[:], sinv[:], 127.0)
            nc.vector.tensor_scalar_mul(scf[:], mx[:], 1.0 / 127.0)
            sc_d = po_d[OLEN : OLEN + SCB].bitcast(F16).rearrange(
                "(p k) -> p k", k=NJ * C)
            nc.sync.dma_start(sc_d, scf[:])
            for c in range(C):
                for p in range(2):
                    for ti, th in enumerate(TT_SIZES):
                        ot = outp.tile([128, F], mybir.dt.int8, tag="ot8",
                                       name="ot8")
                        for j in range(NJ):
                            fj = FSZ[j]
                            col = j * C + c
                            qt = scr.tile([128, 128], FP, tag="qt", name="qt",
                                          bufs=4)
                            nc.scalar.mul(
                                qt[0:fj, 0:th],
                                Y[c][p][0:fj, j * TL + ti * 128 : j * TL + ti * 128 + th],
                                sinv[0:fj, col : col + 1])
                            nc.vector.tensor_scalar(qt[0:fj, 0:th], qt[0:fj, 0:th],
                                                    MAGIC, -MAGIC, AL.add, AL.add)
                            ps = psB.tile([128, 128], FP, tag="tp", name="tp")
                            nc.tensor.transpose(ps[0:th, 0:fj], qt[0:fj, 0:th],
                                                ident[0:fj, 0:fj])
                            nc.scalar.copy(ot[0:th, 128 * j : 128 * j + fj],
                                           ps[0:th, 0:fj])
                        nc.sync.dma_start(yo_d[p, c, ti * 128 : ti * 128 + th, :],
                                          ot[0:th, :])
        else:
            for c in range(C):
                for p in range(2):
                    for ti, th in enumerate(TT_SIZES):
                        ot = outp.tile([128, F], F16, tag="ot", name="ot")
                        for j in range(NJ):
                            fj = FSZ[j]
                            ps = psB.tile([128, 128], FP, tag="tp", name="tp")
                            nc.tensor.transpose(
                                ps[0:th, 0:fj],
                                Y[c][p][0:fj, j * TL + ti * 128 : j * TL + ti * 128 + th],
                                ident[0:fj, 0:fj])
                            nc.scalar.copy(ot[0:th, 128 * j : 128 * j + fj],
                                           ps[0:th, 0:fj])
                        nc.sync.dma_start(yo_d[p, c, ti * 128 : ti * 128 + th, :],
                                          ot[0:th, :])


class _Exec:
    """Cached jitted shard_map executor with output-buffer donation."""

    def __init__(self, nc, devices=None, g=N_CORES):
        import jax
        from jax.sharding import Mesh, PartitionSpec, NamedSharding
        from jax.experimental.shard_map import shard_map
        from concourse.bass2jax import (
            _bass_exec_p, install_neuronx_cc_hook, partition_id_tensor,
        )
        import jax.numpy as jnp

        self.jax = jax
        self.np = np
        install_neuronx_cc_hook()
        partition_name = (nc.partition_id_tensor.name
                          if nc.partition_id_tensor else None)
        in_names, out_names, out_avals = [], [], []
        in_len = None
        for alloc in nc.m.functions[0].allocations:
            if not isinstance(alloc, mybir.MemoryLocationSet):
                continue
            name = alloc.memorylocations[0].name
            if alloc.kind == "ExternalInput":
                if name != partition_name:
                    in_names.append(name)
                    in_len = int(alloc.tensor_shape[0])
            elif alloc.kind == "ExternalOutput":
                out_names.append(name)
                out_avals.append(jax.core.ShapedArray(
                    tuple(alloc.tensor_shape), mybir.dt.np(alloc.dtype)))
        assert in_names == ["pk"] and out_names == ["po"], (in_names, out_names)
        n_params = len(in_names)
        n_outs = len(out_avals)
        all_in = list(in_names) + list(out_names)
        if partition_name is not None:
            all_in.append(partition_name)

        def _bdy(*args):
            operands = list(args)
            if partition_name is not None:
                operands.append(partition_id_tensor())
            return tuple(_bass_exec_p.bind(
                *operands,
                out_avals=tuple(out_avals),
                in_names=tuple(all_in),
                out_names=tuple(out_names),
                lowering_input_output_aliases=(),
                sim_require_finite=True,
                sim_require_nnan=True,
                nc=nc,
            ))

        if devices is None:
            devices = jax.devices()[:g]
        assert len(devices) == g
        self.devices = devices
        self.g = g
        self.per = in_len
        mesh = Mesh(np.asarray(devices), ("core",))
        self.shard = NamedSharding(mesh, PartitionSpec("core"))
        in_specs = (PartitionSpec("core"),) * (n_params + n_outs)
        out_specs = (PartitionSpec("core"),) * n_outs
        jitfn = jax.jit(
            shard_map(_bdy, mesh=mesh, in_specs=in_specs, out_specs=out_specs,
                      check_rep=False),
            donate_argnums=tuple(range(n_params, n_params + n_outs)),
            keep_unused=True,
        )
        aval = out_avals[0]
        gshape = (g * aval.shape[0],)
        gdtype = aval.dtype
        try:
            from concourse.bass2jax import fast_dispatch_compile
            pk_spec = jax.ShapeDtypeStruct((g * self.per,), np.float16,
                                           sharding=self.shard)
            do_spec = jax.ShapeDtypeStruct(gshape, gdtype, sharding=self.shard)
            self.sharded = fast_dispatch_compile(
                lambda: jitfn.lower(pk_spec, do_spec).compile())
        except Exception:
            self.sharded = jitfn
        self.mkzeros = jax.jit(lambda: jnp.zeros(gshape, gdtype),
                               out_shardings=self.shard)
        self.prev_out = None

    def run_packed(self, bufs):
        """bufs: list of g per-core np fp16 buffers (len self.per).
        Returns np (g*OLEN8,) int8 (or f16 in non-OUT8 builds)."""
        jax = self.jax
        donate = self.prev_out if self.prev_out is not None else self.mkzeros()
        shards = [jax.device_put(bufs[k], self.devices[k])
                  for k in range(self.g)]
        pk_dev = jax.make_array_from_single_device_arrays(
            (self.g * self.per,), self.shard, shards)
        (out,) = self.sharded(pk_dev, donate)
        try:
            out.copy_to_host_async()
        except Exception:
            pass
        res = np.asarray(out)
        self.prev_out = out
        return res

    def run(self, inputs):
        """Pack per-core fp16 buffers, pipelining each device_put with the
        next core's packing.  Returns np (N_CORES*OLEN,) fp16."""
        jax = self.jax
        donate = self.prev_out if self.prev_out is not None else self.mkzeros()
        data_real = np.asarray(inputs["data_real"]).reshape(N_CORES, TL, C, F)
        data_imag = np.asarray(inputs["data_imag"]).reshape(N_CORES, TL, C, F)
        wflat = np.empty(8 * WSH, np.float16)
        wflat[GW1 : GW1 + NW] = np.asarray(inputs["W1"]).reshape(NW)
        wflat[GW2 : GW2 + NW] = np.asarray(inputs["W2"]).reshape(NW)
        wflat[GB1 : GB1 + U] = np.asarray(inputs["b1"])
        wflat[GB2 : GB2 + F] = np.asarray(inputs["b2"])
        wflat[GB2 + F :] = 0
        wsh = wflat.reshape(N_CORES, WSH)
        shards = []
        for k in range(N_CORES):
            buf = np.empty(PER, np.float16)
            buf[OFF_XR : OFF_XR + NXV].reshape(TL, C, F)[...] = data_real[k]
            buf[OFF_XI : OFF_XI + NXV].reshape(TL, C, F)[...] = data_imag[k]
            buf[OFF_WS : OFF_WS + WSH] = wsh[k]
            shards.append(jax.device_put(buf, self.devices[k]))
        pk_dev = jax.make_array_from_single_device_arrays(
            (N_CORES * PER,), self.shard, shards)
        (out,) = self.sharded(pk_dev, donate)
        try:
            out.copy_to_host_async()
        except Exception:
            pass
        # pre-fault the host output array while exec+fetch stream in the
        # background (the async transfer runs on C++ threads regardless)
        outbuf = np.empty((C, B, T, F), dtype=np.complex64)
        outbuf.fill(0)
        res = np.asarray(out)
        self.prev_out = out
        return res, outbuf


def _unpack(res, outbuf=None):
    out = outbuf if outbuf is not None else np.empty((C, B, T, F),
                                                     dtype=np.complex64)
    if OUT8:
        g = res.reshape(N_CORES, OLEN8)
        q = g[:, :OLEN].reshape(B, TSPLIT, 2, C, TL, F)
        sc = np.ascontiguousarray(g[:, OLEN:]).view(np.float16)
        sc = sc.reshape(N_CORES, 128, NJ, C)
        scale = np.empty((N_CORES, C, F), np.float32)
        for j in range(NJ):
            fj = FSZ[j]
            scale[:, :, 128 * j : 128 * j + fj] = \
                sc[:, 0:fj, j, :].transpose(0, 2, 1)
        sv = scale.reshape(B, TSPLIT, C, F)
        for b in range(B):
            for th in range(TSPLIT):
                sl = slice(th * TL, (th + 1) * TL)
                s = sv[b, th][:, None, :]              # (C,1,F)
                np.multiply(q[b, th, 0], s, out=out.real[:, b, sl, :])
                np.multiply(q[b, th, 1], s, out=out.imag[:, b, sl, :])
        return out
    g = res.reshape(B, TSPLIT, 2, C, TL, F)   # b, th, p, c, t, f
    for b in range(B):
        for th in range(TSPLIT):
            sl = slice(th * TL, (th + 1) * TL)
            out.real[:, b, sl, :] = g[b, th, 0]
            out.imag[:, b, sl, :] = g[b, th, 1]
    return out


def kernel(**inputs):
    if "ex" not in _CACHED:
        _CACHED["nc"] = _build()
        _CACHED["ex"] = _Exec(_CACHED["nc"])
    ex = _CACHED["ex"]
    res, outbuf = ex.run(inputs)
    return _unpack(res, outbuf)


if __name__ == "__main__":
    rng = np.random.default_rng(0)
    ins = {
        "data_real": rng.standard_normal((B, T, C, F), dtype=np.float32),
        "data_imag": rng.standard_normal((B, T, C, F), dtype=np.float32),
        "ilens": np.full((B,), T, dtype=np.int32),
        "W1": rng.standard_normal((F, U), dtype=np.float32) / np.sqrt(F),
        "b1": np.zeros((U,), dtype=np.float32),
        "W2": rng.standard_normal((U, F), dtype=np.float32) / np.sqrt(U),
        "b2": np.zeros((F,), dtype=np.float32),
    }
    out = kernel(**ins)
    print("kernel ran", out.shape, out.dtype, np.abs(out).mean())
